# revision 49
# baseline (speedup 1.0000x reference)
"""BottleneckAdapter kernel for Trainium2 (Bass/Tile), 8-way data parallel.

out = x + scale * (gelu(LN(x) @ w_down + b_down) @ w_up + b_up)

Strategy per core (2048 tokens of the 16384 total, weights replicated):
  - LN is folded into the down-projection:
        LN(x) @ W' = rstd * (x @ W') - (rstd*mu) * colsum(W') ,
    with W' = norm_w[:,None] * w_down, so the expensive per-element
    normalize pass over D=1024 never happens. mu comes free from the
    cast (ACT Copy with accum_out); sum(x^2) from one DVE
    scalar_tensor_tensor with accum_out.
  - The adapter path runs in bf16 (output is x + 0.001*h, so bf16 error
    in h is ~1e-5 of the output); the residual add is exact fp32.
  - The down-matmul needs x with D on partitions, so x is cast to bf16
    and transposed on-chip with the DMA xbar (dma_start_transpose).
    All 16 transposes are forced into ONE contiguous window (explicit
    deps: after all loads, before all stores) because every
    PASSTHROUGH<->TRANSPOSE xbar-mode transition serializes the SDMA
    engines — interleaved, the DMA streams run at ~1/3 throughput.
  - Engine balance: ACT does cast+rowsum and psum->sbuf preproc copies;
    DVE does x^2/stats smalls, z-corrections and the fp32 residual
    (scalar_tensor_tensor: out = (u*1) + x straight from PSUM); PE does
    down/up matmuls plus the small zh transpose; GpSimd issues the big
    SWDGE loads/stores and preproc scaling so DVE's queue head is free.
"""

import numpy as np

import concourse.bass as bass
import concourse.bacc as bacc
import concourse.mybir as mybir
import concourse.tile as tile
from concourse import bass_utils
from concourse.masks import make_identity
from concourse.tile import add_dep_helper

F32 = mybir.dt.float32
BF16 = mybir.dt.bfloat16
AF = mybir.ActivationFunctionType
OP = mybir.AluOpType

# Problem shapes (hardcoded per the contract).
B, N, D = 4, 4096, 1024
BN = 64                      # bottleneck
N_CORES = 8
TOK_TOTAL = B * N            # 16384
TOK = TOK_TOTAL // N_CORES   # 2048 tokens per core
P = 128                      # partitions
NT = TOK // P                # 16 token tiles per core
NQ = 4                       # quarters (load/store granularity)
TPQ = NT // NQ               # 4 token tiles per quarter
NCH = D // P                 # 8 contraction chunks of 128
EPS = 1e-5
H = D // 2                   # 512 (psum bank half)


def _build_kernel():
    nc = bacc.Bacc(
        "TRN2",
        target_bir_lowering=False,
        debug=False,
        enable_asserts=False,
        num_devices=N_CORES,
    )
    x_d = nc.dram_tensor("x", [TOK, D], F32, kind="ExternalInput")
    nw_d = nc.dram_tensor("norm_w", [D], F32, kind="ExternalInput")
    nb_d = nc.dram_tensor("norm_b", [D], F32, kind="ExternalInput")
    wd_d = nc.dram_tensor("w_down", [D, BN], F32, kind="ExternalInput")
    bd_d = nc.dram_tensor("b_down", [BN], F32, kind="ExternalInput")
    wu_d = nc.dram_tensor("w_up", [BN, D], F32, kind="ExternalInput")
    bu_d = nc.dram_tensor("b_up", [D], F32, kind="ExternalInput")
    sc_d = nc.dram_tensor("scale", [1, 1], F32, kind="ExternalInput")
    out_d = nc.dram_tensor("out", [TOK, D], F32, kind="ExternalOutput")

    with tile.TileContext(nc) as tc:
        _body(
            tc,
            x_d.ap(),
            nw_d.ap(),
            nb_d.ap(),
            wd_d.ap(),
            bd_d.ap(),
            wu_d.ap(),
            bu_d.ap(),
            sc_d.ap(),
            out_d.ap(),
        )
    nc.compile()
    return nc


def _body(tc, x, nw, nb, wd, bd, wu, bu, sc, out):
    from contextlib import ExitStack

    nc = tc.nc
    ctx = ExitStack()
    with ctx:
        x_r = x.rearrange("(t p) d -> p t d", p=P)      # [128, 16, 1024]
        out_r = out.rearrange("(t p) d -> p t d", p=P)

        const = ctx.enter_context(tc.tile_pool(name="const", bufs=1))
        px = ctx.enter_context(tc.tile_pool(name="px", bufs=4))       # x f32 quarters

        # ---------- x loads first: get the SWDGE queue moving immediately ----
        # Each quarter is loaded as two 1MB halves so stats can start early.
        xqs = []
        load_insts = []
        for q in range(NQ):
            xq = px.tile([P, TPQ, D], F32, tag="xq")
            hq = TPQ // 2
            load_insts.append(
                nc.gpsimd.dma_start(
                    out=xq[:, 0:hq, :], in_=x_r[:, q * TPQ : q * TPQ + hq, :]
                )
            )
            load_insts.append(
                nc.gpsimd.dma_start(
                    out=xq[:, hq:TPQ, :], in_=x_r[:, q * TPQ + hq : (q + 1) * TPQ, :]
                )
            )
            xqs.append(xq)

        # ---------- constants / preprocessing ----------
        eps_b = const.tile([P, 1], F32)
        nc.vector.memset(eps_b, EPS)
        ones_col = const.tile([P, 1], BF16)
        nc.vector.memset(ones_col, 1.0)

        # W' = norm_w[:,None] * w_down laid out [p, c, j]; kept fp32 + bf16.
        # All preprocessing element-wise work is kept OFF the vector engine
        # (gpsimd for SBUF-only ops, ACT for PSUM reads) so DVE's queue head
        # is immediately available for the per-tile x^2 stats.
        w_f32 = const.tile([P, NCH, BN], F32)
        nc.sync.dma_start(out=w_f32, in_=wd.rearrange("(c p) j -> p c j", p=P))
        nw_sb = const.tile([P, NCH], F32)
        nc.sync.dma_start(out=nw_sb, in_=nw.rearrange("(c p) -> p c", p=P))
        w_sb = const.tile([P, NCH, BN], BF16)
        for c in range(NCH):
            nc.gpsimd.tensor_scalar_mul(
                w_sb[:, c, :], w_f32[:, c, :], nw_sb[:, c : c + 1]
            )

        ident = const.tile([P, P], F32)
        make_identity(nc, ident)
        ident_bf = const.tile([P, P], BF16)
        make_identity(nc, ident_bf)

        # norm_b laid out [p, c] for the b' matvec.
        nb_sb = const.tile([P, NCH, 1], F32)
        nc.sync.dma_start(out=nb_sb[:, :, 0], in_=nb.rearrange("(c p) -> p c", p=P))
        bd_sb = const.tile([1, BN], BF16)
        nc.gpsimd.dma_start(out=bd_sb, in_=bd[None, :])
        one_1 = const.tile([1, 1], BF16)
        nc.gpsimd.memset(one_1, 1.0)
        ones_row = const.tile([1, P], BF16)
        nc.gpsimd.memset(ones_row, 1.0)

        # w_up_ext = scale * [w_up; b_up]  -> bf16 [65, 1024]
        wue_f = const.tile([BN + 1, D], F32)
        nc.sync.dma_start(out=wue_f[0:BN, :], in_=wu)
        nc.sync.dma_start(out=wue_f[BN : BN + 1, :], in_=bu[None, :])
        sc_b = const.tile([BN + 1, 1], F32)
        nc.gpsimd.dma_start(
            out=sc_b,
            in_=bass.AP(tensor=sc.tensor, offset=0, ap=[[0, BN + 1], [1, 1]]),
        )
        wue = const.tile([BN + 1, D], BF16)
        nc.gpsimd.tensor_scalar_mul(wue, wue_f, sc_b)

        pp = ctx.enter_context(tc.tile_pool(name="pp_psum", bufs=1, space="PSUM"))

        def preproc_rows():
            """s = -colsum(W'); b' = b_down + norm_b @ w_down; broadcast both
            across partitions via K=1 matmuls (one PSUM slot, sequential)."""
            s_ps = pp.tile([1, BN], F32, tag="row")
            for c in range(NCH):
                nc.tensor.matmul(
                    s_ps, ones_col, w_sb[:, c, :], start=(c == 0), stop=(c == NCH - 1)
                )
            s_neg = const.tile([1, BN], BF16)
            nc.scalar.mul(s_neg, s_ps, -1.0)
            rep_ps = pp.tile([P, BN], F32, tag="row")
            nc.tensor.matmul(rep_ps, ones_row, s_neg, start=True, stop=True)
            sneg_r = const.tile([P, BN], F32)
            nc.scalar.copy(sneg_r, rep_ps)

            bp_ps = pp.tile([1, BN], F32, tag="row")
            for c in range(NCH):
                nc.tensor.matmul(
                    bp_ps, nb_sb[:, c, :], w_f32[:, c, :], start=(c == 0), stop=False
                )
            nc.tensor.matmul(bp_ps, one_1, bd_sb, start=False, stop=True)
            b_row = const.tile([1, BN], BF16)
            nc.scalar.copy(b_row, bp_ps)
            rep_ps2 = pp.tile([P, BN], F32, tag="row")
            nc.tensor.matmul(rep_ps2, ones_row, b_row, start=True, stop=True)
            b_rep = const.tile([P, BN], F32)
            nc.scalar.copy(b_rep, rep_ps2)
            return sneg_r, b_rep

        # ---------- main pipeline ----------
        pxb = ctx.enter_context(tc.tile_pool(name="pxb", bufs=16))    # x bf16 tiles
        pxt = ctx.enter_context(tc.tile_pool(name="pxt", bufs=16))    # xT tiles
        pst = ctx.enter_context(tc.tile_pool(name="pst", bufs=8))     # per-tile stats
        psq = ctx.enter_context(tc.tile_pool(name="psq", bufs=2))     # x^2 scratch
        psc = ctx.enter_context(tc.tile_pool(name="psc", bufs=4))     # z-corr temps
        pgt = ctx.enter_context(tc.tile_pool(name="pgt", bufs=4))     # gT tiles
        pout = ctx.enter_context(tc.tile_pool(name="pout", bufs=2))   # out staging
        zps = ctx.enter_context(tc.tile_pool(name="zps", bufs=2, space="PSUM"))
        ztps = ctx.enter_context(tc.tile_pool(name="ztps", bufs=1, space="PSUM"))
        ups = ctx.enter_context(tc.tile_pool(name="ups", bufs=3, space="PSUM"))

        state = {}

        def phase_cast(q):
            """cast + row-sum (one fused ACT Copy w/ accum_out) per tile."""
            xq = xqs[q]
            sumx = pst.tile([P, TPQ], F32, tag="sumx")
            xbs = []
            for i in range(TPQ):
                xb = pxb.tile([P, D], BF16, tag="xb")
                nc.scalar.activation(
                    xb, xq[:, i, :], AF.Copy, accum_out=sumx[:, i : i + 1]
                )
                xbs.append(xb)
            state[q] = (sumx, xbs)

        def phase_a(q):
            """sum-of-squares + stats + transposes for quarter q."""
            sumx, xbs = state[q]
            sumsq = pst.tile([P, TPQ], F32, tag="sumsq")
            xts = []
            for i in range(TPQ):
                x2 = psq.tile([P, D], BF16, tag="x2")
                nc.vector.scalar_tensor_tensor(
                    out=x2,
                    in0=xbs[i],
                    scalar=1.0,
                    in1=xbs[i],
                    op0=OP.mult,
                    op1=OP.mult,
                    accum_out=sumsq[:, i : i + 1],
                )
                # contiguous 2D transpose output (xbar fast path):
                # xt_i[p, c*128 + t] = xb_i[t, c*128 + p]
                xt_i = pxt.tile([P, D], BF16, tag="xt")
                tp_insts.append(
                    nc.sync.dma_start_transpose(
                        out=xt_i.rearrange("p (c t) -> p c t", t=P), in_=xbs[i]
                    )
                )
                xts.append(xt_i)
            # mu = sumx/D ; var = sumsq/D - mu^2 ; rstd = 1/sqrt(var+eps)
            mu_q = pst.tile([P, TPQ], F32, tag="mu")
            nc.vector.tensor_scalar_mul(mu_q, sumx, 1.0 / D)
            musq = pst.tile([P, TPQ], F32, tag="musq")
            nc.vector.tensor_mul(musq, mu_q, mu_q)
            var_q = pst.tile([P, TPQ], F32, tag="var")
            nc.vector.scalar_tensor_tensor(
                out=var_q, in0=sumsq, scalar=1.0 / D, in1=musq,
                op0=OP.mult, op1=OP.subtract,
            )
            srt = pst.tile([P, TPQ], F32, tag="srt")
            nc.scalar.activation(srt, var_q, AF.Sqrt, bias=eps_b)
            rstd_q = pst.tile([P, TPQ], F32, tag="rstd")
            nc.vector.reciprocal(rstd_q, srt)
            mr_q = pst.tile([P, TPQ], F32, tag="mr")
            nc.vector.tensor_mul(mr_q, mu_q, rstd_q)
            state[q] = (xts, rstd_q, mr_q)

        def phase_b(q):
            """matmuls + gelu + residual + store for quarter q."""
            xq = xqs[q]
            xts, rstd_q, mr_q = state.pop(q)
            ot = pout.tile([P, TPQ, D], F32, tag="ot")
            for i in range(TPQ):
                rstd = rstd_q[:, i : i + 1]
                mr = mr_q[:, i : i + 1]
                z = zps.tile([P, BN], F32, tag="z")
                for c in range(NCH):
                    nc.tensor.matmul(
                        z,
                        xts[i][:, c * P : (c + 1) * P],
                        w_sb[:, c, :],
                        start=(c == 0),
                        stop=(c == NCH - 1),
                    )
                # zh = rstd*z + ((-s)*(mu*rstd) + b')
                t3 = psc.tile([P, BN], F32, tag="t3")
                nc.vector.scalar_tensor_tensor(
                    out=t3, in0=sneg_r, scalar=mr, in1=b_rep, op0=OP.mult, op1=OP.add
                )
                zh = psc.tile([P, BN], F32, tag="zh")
                nc.vector.scalar_tensor_tensor(
                    out=zh, in0=z, scalar=rstd, in1=t3, op0=OP.mult, op1=OP.add
                )
                # transpose zh -> [64, 128], gelu into gT rows 0..63, ones row 64
                zt = ztps.tile([BN, P], F32, tag="zt")
                nc.tensor.transpose(zt, zh, ident)
                gt = pgt.tile([BN + 1, P], BF16, tag="gt")
                nc.scalar.activation(gt[0:BN, :], zt, AF.Gelu)
                nc.vector.memset(gt[BN : BN + 1, :], 1.0)
                # up-projection (+ scaled bias via the ones row), then
                # residual out = 1.0*u + x (fp32), per 512-wide half.
                for h in range(2):
                    u = ups.tile([P, H], F32, tag="u")
                    nc.tensor.matmul(
                        u, gt, wue[:, h * H : (h + 1) * H], start=True, stop=True
                    )
                    nc.vector.scalar_tensor_tensor(
                        out=ot[:, i, h * H : (h + 1) * H],
                        in0=u,
                        scalar=1.0,
                        in1=xq[:, i, h * H : (h + 1) * H],
                        op0=OP.mult,
                        op1=OP.add,
                    )
            hq = TPQ // 2
            store_insts.append(
                nc.gpsimd.dma_start(
                    out=out_r[:, q * TPQ : q * TPQ + hq, :], in_=ot[:, 0:hq, :]
                )
            )
            store_insts.append(
                nc.gpsimd.dma_start(
                    out=out_r[:, q * TPQ + hq : (q + 1) * TPQ, :], in_=ot[:, hq:TPQ, :]
                )
            )

        # All casts first (ACT FIFO unblocked), then all stats+transposes
        # (one contiguous xbar window), then all compute/store B phases.
        tp_insts = []
        store_insts = []
        for q in range(NQ):
            phase_cast(q)
        sneg_r, b_rep = preproc_rows()
        for q in range(NQ):
            phase_a(q)
        for q in range(NQ):
            phase_b(q)

        # Force a single xbar window: every transpose after ALL loads, every
        # store after the LAST transpose. Otherwise the scheduler interleaves
        # copies and transposes and every mode transition serializes the
        # SDMA engines.
        for tp in tp_insts:
            for ld in load_insts:
                add_dep_helper(tp.ins, ld.ins, reason="xbar window: after loads")
        for st in store_insts:
            add_dep_helper(st.ins, tp_insts[-1].ins, reason="xbar window: stores after")


_NC = None


def _get_nc():
    global _NC
    if _NC is None:
        _NC = _build_kernel()
    return _NC


def _make_in_maps(inputs):
    x = np.ascontiguousarray(np.asarray(inputs["x"], dtype=np.float32)).reshape(
        TOK_TOTAL, D
    )
    shared = {
        "norm_w": np.ascontiguousarray(np.asarray(inputs["norm_w"], np.float32)),
        "norm_b": np.ascontiguousarray(np.asarray(inputs["norm_b"], np.float32)),
        "w_down": np.ascontiguousarray(np.asarray(inputs["w_down"], np.float32)),
        "b_down": np.ascontiguousarray(np.asarray(inputs["b_down"], np.float32)),
        "w_up": np.ascontiguousarray(np.asarray(inputs["w_up"], np.float32)),
        "b_up": np.ascontiguousarray(np.asarray(inputs["b_up"], np.float32)),
        "scale": np.asarray(inputs["scale"], np.float32).reshape(1, 1),
    }
    in_maps = []
    for c in range(N_CORES):
        m = dict(shared)
        m["x"] = np.ascontiguousarray(x[c * TOK : (c + 1) * TOK])
        in_maps.append(m)
    return in_maps


def run(inputs, trace=False, **kwargs):
    nc = _get_nc()
    in_maps = _make_in_maps(inputs)
    res = bass_utils.run_bass_kernel_spmd(
        nc, in_maps, core_ids=list(range(N_CORES)), trace=trace, **kwargs
    )
    shards = [res.results[c]["out"] for c in range(N_CORES)]
    full = np.concatenate(shards, axis=0).reshape(B, N, D).astype(np.float32)
    return full, res


def kernel(**inputs):
    full, _ = run(inputs, trace=False)
    return full


# revision 51
# speedup vs baseline: 1.0324x; 1.0324x over previous
"""BottleneckAdapter kernel for Trainium2 (Bass/Tile), 8-way data parallel.

out = x + scale * (gelu(LN(x) @ w_down + b_down) @ w_up + b_up)

Strategy per core (2048 tokens of the 16384 total, weights replicated):
  - LN is folded into the down-projection:
        LN(x) @ W' = rstd * (x @ W') - (rstd*mu) * colsum(W') ,
    with W' = norm_w[:,None] * w_down, so the expensive per-element
    normalize pass over D=1024 never happens. mu comes free from the
    cast (ACT Copy with accum_out); sum(x^2) from one DVE
    scalar_tensor_tensor with accum_out.
  - The adapter path runs in bf16 (output is x + 0.001*h, so bf16 error
    in h is ~1e-5 of the output); the residual add is exact fp32.
  - The down-matmul needs x with D on partitions, so x is cast to bf16
    and transposed on-chip with the DMA xbar (dma_start_transpose).
    All 16 transposes are forced into ONE contiguous window (explicit
    deps: after all loads, before all stores) because every
    PASSTHROUGH<->TRANSPOSE xbar-mode transition serializes the SDMA
    engines — interleaved, the DMA streams run at ~1/3 throughput.
  - Engine balance: ACT does cast+rowsum and psum->sbuf preproc copies;
    DVE does x^2/stats smalls, z-corrections and the fp32 residual
    (scalar_tensor_tensor: out = (u*1) + x straight from PSUM); PE does
    down/up matmuls plus the small zh transpose; GpSimd issues the big
    SWDGE loads/stores and preproc scaling so DVE's queue head is free.
"""

import numpy as np

import concourse.bass as bass
import concourse.bacc as bacc
import concourse.mybir as mybir
import concourse.tile as tile
from concourse import bass_utils
from concourse.masks import make_identity
from concourse.tile import add_dep_helper

F32 = mybir.dt.float32
BF16 = mybir.dt.bfloat16
AF = mybir.ActivationFunctionType
OP = mybir.AluOpType

# Problem shapes (hardcoded per the contract).
B, N, D = 4, 4096, 1024
BN = 64                      # bottleneck
N_CORES = 8
TOK_TOTAL = B * N            # 16384
TOK = TOK_TOTAL // N_CORES   # 2048 tokens per core
P = 128                      # partitions
NT = TOK // P                # 16 token tiles per core
NQ = 4                       # quarters (load/store granularity)
TPQ = NT // NQ               # 4 token tiles per quarter
NCH = D // P                 # 8 contraction chunks of 128
EPS = 1e-5
H = D // 2                   # 512 (psum bank half)


def _build_kernel():
    nc = bacc.Bacc(
        "TRN2",
        target_bir_lowering=False,
        debug=False,
        enable_asserts=False,
        num_devices=N_CORES,
    )
    x_d = nc.dram_tensor("x", [TOK, D], F32, kind="ExternalInput")
    nw_d = nc.dram_tensor("norm_w", [D], F32, kind="ExternalInput")
    nb_d = nc.dram_tensor("norm_b", [D], F32, kind="ExternalInput")
    wd_d = nc.dram_tensor("w_down", [D, BN], F32, kind="ExternalInput")
    bd_d = nc.dram_tensor("b_down", [BN], F32, kind="ExternalInput")
    wu_d = nc.dram_tensor("w_up", [BN, D], F32, kind="ExternalInput")
    bu_d = nc.dram_tensor("b_up", [D], F32, kind="ExternalInput")
    sc_d = nc.dram_tensor("scale", [1, 1], F32, kind="ExternalInput")
    out_d = nc.dram_tensor("out", [TOK, D], F32, kind="ExternalOutput")

    with tile.TileContext(nc) as tc:
        _body(
            tc,
            x_d.ap(),
            nw_d.ap(),
            nb_d.ap(),
            wd_d.ap(),
            bd_d.ap(),
            wu_d.ap(),
            bu_d.ap(),
            sc_d.ap(),
            out_d.ap(),
        )
    nc.compile()
    return nc


def _body(tc, x, nw, nb, wd, bd, wu, bu, sc, out):
    from contextlib import ExitStack

    nc = tc.nc
    ctx = ExitStack()
    with ctx:
        x_r = x.rearrange("(t p) d -> p t d", p=P)      # [128, 16, 1024]
        out_r = out.rearrange("(t p) d -> p t d", p=P)

        const = ctx.enter_context(tc.tile_pool(name="const", bufs=1))
        px = ctx.enter_context(tc.tile_pool(name="px", bufs=4))       # x f32 quarters

        # ---------- x loads first: get the SWDGE queue moving immediately ----
        # Each quarter is loaded as two 1MB halves so stats can start early.
        xqs = []
        load_insts = []
        for q in range(NQ):
            xq = px.tile([P, TPQ, D], F32, tag="xq")
            hq = TPQ // 2
            load_insts.append(
                nc.gpsimd.dma_start(
                    out=xq[:, 0:hq, :], in_=x_r[:, q * TPQ : q * TPQ + hq, :]
                )
            )
            load_insts.append(
                nc.gpsimd.dma_start(
                    out=xq[:, hq:TPQ, :], in_=x_r[:, q * TPQ + hq : (q + 1) * TPQ, :]
                )
            )
            xqs.append(xq)

        # ---------- constants / preprocessing ----------
        eps_b = const.tile([P, 1], F32)
        nc.vector.memset(eps_b, EPS)
        ones_col = const.tile([P, 1], BF16)
        nc.vector.memset(ones_col, 1.0)

        # W' = norm_w[:,None] * w_down laid out [p, c, j]; kept fp32 + bf16.
        # All preprocessing element-wise work is kept OFF the vector engine
        # (gpsimd for SBUF-only ops, ACT for PSUM reads) so DVE's queue head
        # is immediately available for the per-tile x^2 stats.
        w_f32 = const.tile([P, NCH, BN], F32)
        nc.sync.dma_start(out=w_f32, in_=wd.rearrange("(c p) j -> p c j", p=P))
        nw_sb = const.tile([P, NCH], F32)
        nc.sync.dma_start(out=nw_sb, in_=nw.rearrange("(c p) -> p c", p=P))
        w_sb = const.tile([P, NCH, BN], BF16)
        for c in range(NCH):
            nc.gpsimd.tensor_scalar_mul(
                w_sb[:, c, :], w_f32[:, c, :], nw_sb[:, c : c + 1]
            )

        ident = const.tile([P, P], F32)
        make_identity(nc, ident)
        ident_bf = const.tile([P, P], BF16)
        make_identity(nc, ident_bf)

        # norm_b laid out [p, c] for the b' matvec.
        nb_sb = const.tile([P, NCH, 1], F32)
        nc.sync.dma_start(out=nb_sb[:, :, 0], in_=nb.rearrange("(c p) -> p c", p=P))
        bd_sb = const.tile([1, BN], BF16)
        nc.gpsimd.dma_start(out=bd_sb, in_=bd[None, :])
        one_1 = const.tile([1, 1], BF16)
        nc.gpsimd.memset(one_1, 1.0)
        ones_row = const.tile([1, P], BF16)
        nc.gpsimd.memset(ones_row, 1.0)

        # w_up_ext = scale * [w_up; b_up]  -> bf16 [65, 1024]
        wue_f = const.tile([BN + 1, D], F32)
        nc.sync.dma_start(out=wue_f[0:BN, :], in_=wu)
        nc.sync.dma_start(out=wue_f[BN : BN + 1, :], in_=bu[None, :])
        sc_b = const.tile([BN + 1, 1], F32)
        nc.gpsimd.dma_start(
            out=sc_b,
            in_=bass.AP(tensor=sc.tensor, offset=0, ap=[[0, BN + 1], [1, 1]]),
        )
        wue = const.tile([BN + 1, D], BF16)
        nc.gpsimd.tensor_scalar_mul(wue, wue_f, sc_b)

        pp = ctx.enter_context(tc.tile_pool(name="pp_psum", bufs=1, space="PSUM"))

        def preproc_rows():
            """s = -colsum(W'); b' = b_down + norm_b @ w_down; broadcast both
            across partitions via K=1 matmuls (one PSUM slot, sequential)."""
            s_ps = pp.tile([1, BN], F32, tag="row")
            for c in range(NCH):
                nc.tensor.matmul(
                    s_ps, ones_col, w_sb[:, c, :], start=(c == 0), stop=(c == NCH - 1)
                )
            s_neg = const.tile([1, BN], BF16)
            nc.scalar.mul(s_neg, s_ps, -1.0)
            rep_ps = pp.tile([P, BN], F32, tag="row")
            nc.tensor.matmul(rep_ps, ones_row, s_neg, start=True, stop=True)
            sneg_r = const.tile([P, BN], F32)
            nc.scalar.copy(sneg_r, rep_ps)

            bp_ps = pp.tile([1, BN], F32, tag="row")
            for c in range(NCH):
                nc.tensor.matmul(
                    bp_ps, nb_sb[:, c, :], w_f32[:, c, :], start=(c == 0), stop=False
                )
            nc.tensor.matmul(bp_ps, one_1, bd_sb, start=False, stop=True)
            b_row = const.tile([1, BN], BF16)
            nc.scalar.copy(b_row, bp_ps)
            rep_ps2 = pp.tile([P, BN], F32, tag="row")
            nc.tensor.matmul(rep_ps2, ones_row, b_row, start=True, stop=True)
            b_rep = const.tile([P, BN], F32)
            nc.scalar.copy(b_rep, rep_ps2)
            return sneg_r, b_rep

        # ---------- main pipeline ----------
        pxb = ctx.enter_context(tc.tile_pool(name="pxb", bufs=16))    # x bf16 tiles
        pxt = ctx.enter_context(tc.tile_pool(name="pxt", bufs=16))    # xT tiles
        pst = ctx.enter_context(tc.tile_pool(name="pst", bufs=8))     # per-tile stats
        psq = ctx.enter_context(tc.tile_pool(name="psq", bufs=2))     # x^2 scratch
        psc = ctx.enter_context(tc.tile_pool(name="psc", bufs=4))     # z-corr temps
        pgt = ctx.enter_context(tc.tile_pool(name="pgt", bufs=4))     # gT tiles
        pout = ctx.enter_context(tc.tile_pool(name="pout", bufs=2))   # out staging
        zps = ctx.enter_context(tc.tile_pool(name="zps", bufs=2, space="PSUM"))
        ztps = ctx.enter_context(tc.tile_pool(name="ztps", bufs=1, space="PSUM"))
        ups = ctx.enter_context(tc.tile_pool(name="ups", bufs=3, space="PSUM"))

        state = {}

        def phase_cast(q):
            """cast + row-sum (one fused ACT Copy w/ accum_out) per tile."""
            xq = xqs[q]
            sumx = pst.tile([P, TPQ], F32, tag="sumx")
            xbs = []
            for i in range(TPQ):
                xb = pxb.tile([P, D], BF16, tag="xb")
                nc.scalar.activation(
                    xb, xq[:, i, :], AF.Copy, accum_out=sumx[:, i : i + 1]
                )
                xbs.append(xb)
            state[q] = (sumx, xbs)

        def phase_a(q):
            """sum-of-squares + stats + transposes for quarter q."""
            sumx, xbs = state[q]
            sumsq = pst.tile([P, TPQ], F32, tag="sumsq")
            xts = []
            for i in range(TPQ):
                x2 = psq.tile([P, D], BF16, tag="x2")
                nc.vector.scalar_tensor_tensor(
                    out=x2,
                    in0=xbs[i],
                    scalar=1.0,
                    in1=xbs[i],
                    op0=OP.mult,
                    op1=OP.mult,
                    accum_out=sumsq[:, i : i + 1],
                )
                # contiguous 2D transpose output (xbar fast path):
                # xt_i[p, c*128 + t] = xb_i[t, c*128 + p]
                xt_i = pxt.tile([P, D], BF16, tag="xt")
                tp_insts.append(
                    nc.sync.dma_start_transpose(
                        out=xt_i.rearrange("p (c t) -> p c t", t=P), in_=xbs[i]
                    )
                )
                xts.append(xt_i)
            # mu = sumx/D ; var = sumsq/D - mu^2 ; rstd = 1/sqrt(var+eps)
            mu_q = pst.tile([P, TPQ], F32, tag="mu")
            nc.vector.tensor_scalar_mul(mu_q, sumx, 1.0 / D)
            musq = pst.tile([P, TPQ], F32, tag="musq")
            nc.vector.tensor_mul(musq, mu_q, mu_q)
            var_q = pst.tile([P, TPQ], F32, tag="var")
            nc.vector.scalar_tensor_tensor(
                out=var_q, in0=sumsq, scalar=1.0 / D, in1=musq,
                op0=OP.mult, op1=OP.subtract,
            )
            srt = pst.tile([P, TPQ], F32, tag="srt")
            nc.scalar.activation(srt, var_q, AF.Sqrt, bias=eps_b)
            rstd_q = pst.tile([P, TPQ], F32, tag="rstd")
            nc.vector.reciprocal(rstd_q, srt)
            mr_q = pst.tile([P, TPQ], F32, tag="mr")
            nc.vector.tensor_mul(mr_q, mu_q, rstd_q)
            state[q] = (xts, rstd_q, mr_q)

        def phase_b(q):
            """matmuls + gelu + residual + store for quarter q."""
            xq = xqs[q]
            xts, rstd_q, mr_q = state.pop(q)
            ot = pout.tile([P, TPQ, D], F32, tag="ot")
            for i in range(TPQ):
                rstd = rstd_q[:, i : i + 1]
                mr = mr_q[:, i : i + 1]
                z = zps.tile([P, BN], F32, tag="z")
                for c in range(NCH):
                    nc.tensor.matmul(
                        z,
                        xts[i][:, c * P : (c + 1) * P],
                        w_sb[:, c, :],
                        start=(c == 0),
                        stop=(c == NCH - 1),
                    )
                # zh = rstd*z + ((-s)*(mu*rstd) + b')
                t3 = psc.tile([P, BN], F32, tag="t3")
                nc.vector.scalar_tensor_tensor(
                    out=t3, in0=sneg_r, scalar=mr, in1=b_rep, op0=OP.mult, op1=OP.add
                )
                zh = psc.tile([P, BN], F32, tag="zh")
                nc.vector.scalar_tensor_tensor(
                    out=zh, in0=z, scalar=rstd, in1=t3, op0=OP.mult, op1=OP.add
                )
                # transpose zh -> [64, 128], gelu into gT rows 0..63, ones row 64
                zt = ztps.tile([BN, P], F32, tag="zt")
                nc.tensor.transpose(zt, zh, ident)
                gt = pgt.tile([BN + 1, P], BF16, tag="gt")
                nc.scalar.activation(gt[0:BN, :], zt, AF.Gelu)
                nc.vector.memset(gt[BN : BN + 1, :], 1.0)
                # up-projection (+ scaled bias via the ones row), then
                # residual out = 1.0*u + x (fp32), per 512-wide half.
                for h in range(2):
                    u = ups.tile([P, H], F32, tag="u")
                    nc.tensor.matmul(
                        u, gt, wue[:, h * H : (h + 1) * H], start=True, stop=True
                    )
                    nc.vector.scalar_tensor_tensor(
                        out=ot[:, i, h * H : (h + 1) * H],
                        in0=u,
                        scalar=1.0,
                        in1=xq[:, i, h * H : (h + 1) * H],
                        op0=OP.mult,
                        op1=OP.add,
                    )
            hq = TPQ // 2
            store_insts.append(
                nc.gpsimd.dma_start(
                    out=out_r[:, q * TPQ : q * TPQ + hq, :], in_=ot[:, 0:hq, :]
                )
            )
            store_insts.append(
                nc.gpsimd.dma_start(
                    out=out_r[:, q * TPQ + hq : (q + 1) * TPQ, :], in_=ot[:, hq:TPQ, :]
                )
            )

        # All casts first (ACT FIFO unblocked), then all stats+transposes
        # (one contiguous xbar window), then all compute/store B phases.
        tp_insts = []
        store_insts = []
        for q in range(NQ):
            phase_cast(q)
        sneg_r, b_rep = preproc_rows()
        for q in range(NQ):
            phase_a(q)
        for q in range(NQ):
            phase_b(q)

        # Force a single xbar window: every transpose after ALL loads, every
        # store after the LAST transpose. Otherwise the scheduler interleaves
        # copies and transposes and every mode transition serializes the
        # SDMA engines.
        for tp in tp_insts:
            for ld in load_insts:
                add_dep_helper(tp.ins, ld.ins, reason="xbar window: after loads")
        for st in store_insts:
            add_dep_helper(st.ins, tp_insts[-1].ins, reason="xbar window: stores after")


_NC = None


def _get_nc():
    global _NC
    if _NC is None:
        _NC = _build_kernel()
    return _NC


def _make_in_maps(inputs):
    x = np.ascontiguousarray(np.asarray(inputs["x"], dtype=np.float32)).reshape(
        TOK_TOTAL, D
    )
    shared = {
        "norm_w": np.ascontiguousarray(np.asarray(inputs["norm_w"], np.float32)),
        "norm_b": np.ascontiguousarray(np.asarray(inputs["norm_b"], np.float32)),
        "w_down": np.ascontiguousarray(np.asarray(inputs["w_down"], np.float32)),
        "b_down": np.ascontiguousarray(np.asarray(inputs["b_down"], np.float32)),
        "w_up": np.ascontiguousarray(np.asarray(inputs["w_up"], np.float32)),
        "b_up": np.ascontiguousarray(np.asarray(inputs["b_up"], np.float32)),
        "scale": np.asarray(inputs["scale"], np.float32).reshape(1, 1),
    }
    in_maps = []
    for c in range(N_CORES):
        m = dict(shared)
        m["x"] = np.ascontiguousarray(x[c * TOK : (c + 1) * TOK])
        in_maps.append(m)
    return in_maps


def run(inputs, trace=False, **kwargs):
    nc = _get_nc()
    in_maps = _make_in_maps(inputs)
    res = bass_utils.run_bass_kernel_spmd(
        nc, in_maps, core_ids=list(range(N_CORES)), trace=trace, **kwargs
    )
    shards = [res.results[c]["out"] for c in range(N_CORES)]
    full = np.concatenate(shards, axis=0).reshape(B, N, D).astype(np.float32)
    return full, res


def kernel(**inputs):
    full, _ = run(inputs, trace=False)
    return full


# revision 53
# speedup vs baseline: 1.0512x; 1.0182x over previous
"""BottleneckAdapter kernel for Trainium2 (Bass/Tile), 8-way data parallel.

out = x + scale * (gelu(LN(x) @ w_down + b_down) @ w_up + b_up)

Strategy per core (2048 tokens of the 16384 total, weights replicated):
  - LN is folded into the down-projection:
        LN(x) @ W' = rstd * (x @ W') - (rstd*mu) * colsum(W') ,
    with W' = norm_w[:,None] * w_down, so the expensive per-element
    normalize pass over D=1024 never happens. mu comes free from the
    cast (ACT Copy with accum_out); sum(x^2) from one DVE
    scalar_tensor_tensor with accum_out.
  - The adapter path runs in bf16 (output is x + 0.001*h, so bf16 error
    in h is ~1e-5 of the output); the residual add is exact fp32.
  - The down-matmul needs x with D on partitions, so x is cast to bf16
    and transposed on-chip with the DMA xbar (dma_start_transpose).
    All 16 transposes are forced into ONE contiguous window (explicit
    deps: after all loads, before all stores) because every
    PASSTHROUGH<->TRANSPOSE xbar-mode transition serializes the SDMA
    engines — interleaved, the DMA streams run at ~1/3 throughput.
  - Engine balance: ACT does cast+rowsum and psum->sbuf preproc copies;
    DVE does x^2/stats smalls, z-corrections and the fp32 residual
    (scalar_tensor_tensor: out = (u*1) + x straight from PSUM); PE does
    down/up matmuls plus the small zh transpose; GpSimd issues the big
    SWDGE loads/stores and preproc scaling so DVE's queue head is free.
"""

import numpy as np

import concourse.bass as bass
import concourse.bacc as bacc
import concourse.mybir as mybir
import concourse.tile as tile
from concourse import bass_utils
from concourse.masks import make_identity
from concourse.tile import add_dep_helper

F32 = mybir.dt.float32
BF16 = mybir.dt.bfloat16
AF = mybir.ActivationFunctionType
OP = mybir.AluOpType

# Problem shapes (hardcoded per the contract).
B, N, D = 4, 4096, 1024
BN = 64                      # bottleneck
N_CORES = 8
TOK_TOTAL = B * N            # 16384
TOK = TOK_TOTAL // N_CORES   # 2048 tokens per core
P = 128                      # partitions
NT = TOK // P                # 16 token tiles per core
NQ = 4                       # quarters (load/store granularity)
TPQ = NT // NQ               # 4 token tiles per quarter
NCH = D // P                 # 8 contraction chunks of 128
EPS = 1e-5
H = D // 2                   # 512 (psum bank half)


def _build_kernel():
    nc = bacc.Bacc(
        "TRN2",
        target_bir_lowering=False,
        debug=False,
        enable_asserts=False,
        num_devices=N_CORES,
    )
    x_d = nc.dram_tensor("x", [TOK, D], F32, kind="ExternalInput")
    nw_d = nc.dram_tensor("norm_w", [D], F32, kind="ExternalInput")
    nb_d = nc.dram_tensor("norm_b", [D], F32, kind="ExternalInput")
    wd_d = nc.dram_tensor("w_down", [D, BN], F32, kind="ExternalInput")
    bd_d = nc.dram_tensor("b_down", [BN], F32, kind="ExternalInput")
    wu_d = nc.dram_tensor("w_up", [BN, D], F32, kind="ExternalInput")
    bu_d = nc.dram_tensor("b_up", [D], F32, kind="ExternalInput")
    sc_d = nc.dram_tensor("scale", [1, 1], F32, kind="ExternalInput")
    out_d = nc.dram_tensor("out", [TOK, D], F32, kind="ExternalOutput")

    with tile.TileContext(nc) as tc:
        _body(
            tc,
            x_d.ap(),
            nw_d.ap(),
            nb_d.ap(),
            wd_d.ap(),
            bd_d.ap(),
            wu_d.ap(),
            bu_d.ap(),
            sc_d.ap(),
            out_d.ap(),
        )
    nc.compile()
    return nc


def _body(tc, x, nw, nb, wd, bd, wu, bu, sc, out):
    from contextlib import ExitStack

    nc = tc.nc
    ctx = ExitStack()
    with ctx:
        x_r = x.rearrange("(t p) d -> p t d", p=P)      # [128, 16, 1024]
        out_r = out.rearrange("(t p) d -> p t d", p=P)

        const = ctx.enter_context(tc.tile_pool(name="const", bufs=1))
        px = ctx.enter_context(tc.tile_pool(name="px", bufs=4))       # x f32 quarters

        # ---------- x loads first: get the SWDGE queue moving immediately ----
        # Each quarter is loaded as two 1MB halves so stats can start early.
        xqs = []
        load_insts = []
        for q in range(NQ):
            xq = px.tile([P, TPQ, D], F32, tag="xq")
            hq = TPQ // 2
            load_insts.append(
                nc.gpsimd.dma_start(
                    out=xq[:, 0:hq, :], in_=x_r[:, q * TPQ : q * TPQ + hq, :]
                )
            )
            load_insts.append(
                nc.gpsimd.dma_start(
                    out=xq[:, hq:TPQ, :], in_=x_r[:, q * TPQ + hq : (q + 1) * TPQ, :]
                )
            )
            xqs.append(xq)

        # ---------- constants / preprocessing ----------
        eps_b = const.tile([P, 1], F32)
        nc.vector.memset(eps_b, EPS)
        ones_col = const.tile([P, 1], BF16)
        nc.vector.memset(ones_col, 1.0)

        # W' = norm_w[:,None] * w_down laid out [p, c, j]; kept fp32 + bf16.
        # All preprocessing element-wise work is kept OFF the vector engine
        # (gpsimd for SBUF-only ops, ACT for PSUM reads) so DVE's queue head
        # is immediately available for the per-tile x^2 stats.
        w_f32 = const.tile([P, NCH, BN], F32)
        nc.sync.dma_start(out=w_f32, in_=wd.rearrange("(c p) j -> p c j", p=P))
        nw_sb = const.tile([P, NCH], F32)
        nc.sync.dma_start(out=nw_sb, in_=nw.rearrange("(c p) -> p c", p=P))
        w_sb = const.tile([P, NCH, BN], BF16)
        for c in range(NCH):
            nc.gpsimd.tensor_scalar_mul(
                w_sb[:, c, :], w_f32[:, c, :], nw_sb[:, c : c + 1]
            )

        ident = const.tile([P, P], F32)
        make_identity(nc, ident)
        ident_bf = const.tile([P, P], BF16)
        make_identity(nc, ident_bf)

        # norm_b laid out [p, c] for the b' matvec.
        nb_sb = const.tile([P, NCH, 1], F32)
        nc.sync.dma_start(out=nb_sb[:, :, 0], in_=nb.rearrange("(c p) -> p c", p=P))
        bd_sb = const.tile([1, BN], BF16)
        nc.gpsimd.dma_start(out=bd_sb, in_=bd[None, :])
        one_1 = const.tile([1, 1], BF16)
        nc.gpsimd.memset(one_1, 1.0)
        ones_row = const.tile([1, P], BF16)
        nc.gpsimd.memset(ones_row, 1.0)

        # w_up_ext = scale * [w_up; b_up]  -> bf16 [65, 1024]
        wue_f = const.tile([BN + 1, D], F32)
        nc.sync.dma_start(out=wue_f[0:BN, :], in_=wu)
        nc.sync.dma_start(out=wue_f[BN : BN + 1, :], in_=bu[None, :])
        sc_b = const.tile([BN + 1, 1], F32)
        nc.gpsimd.dma_start(
            out=sc_b,
            in_=bass.AP(tensor=sc.tensor, offset=0, ap=[[0, BN + 1], [1, 1]]),
        )
        wue = const.tile([BN + 1, D], BF16)
        nc.gpsimd.tensor_scalar_mul(wue, wue_f, sc_b)

        pp = ctx.enter_context(tc.tile_pool(name="pp_psum", bufs=1, space="PSUM"))

        def preproc_rows():
            """s = -colsum(W'); b' = b_down + norm_b @ w_down; broadcast both
            across partitions via K=1 matmuls (one PSUM slot, sequential)."""
            s_ps = pp.tile([1, BN], F32, tag="row")
            for c in range(NCH):
                nc.tensor.matmul(
                    s_ps, ones_col, w_sb[:, c, :], start=(c == 0), stop=(c == NCH - 1)
                )
            s_neg = const.tile([1, BN], BF16)
            nc.scalar.mul(s_neg, s_ps, -1.0)
            rep_ps = pp.tile([P, BN], F32, tag="row")
            nc.tensor.matmul(rep_ps, ones_row, s_neg, start=True, stop=True)
            sneg_r = const.tile([P, BN], F32)
            nc.scalar.copy(sneg_r, rep_ps)

            bp_ps = pp.tile([1, BN], F32, tag="row")
            for c in range(NCH):
                nc.tensor.matmul(
                    bp_ps, nb_sb[:, c, :], w_f32[:, c, :], start=(c == 0), stop=False
                )
            nc.tensor.matmul(bp_ps, one_1, bd_sb, start=False, stop=True)
            b_row = const.tile([1, BN], BF16)
            nc.scalar.copy(b_row, bp_ps)
            rep_ps2 = pp.tile([P, BN], F32, tag="row")
            nc.tensor.matmul(rep_ps2, ones_row, b_row, start=True, stop=True)
            b_rep = const.tile([P, BN], F32)
            nc.scalar.copy(b_rep, rep_ps2)
            return sneg_r, b_rep

        # ---------- main pipeline ----------
        pxb = ctx.enter_context(tc.tile_pool(name="pxb", bufs=16))    # x bf16 tiles
        pxt = ctx.enter_context(tc.tile_pool(name="pxt", bufs=16))    # xT tiles
        pst = ctx.enter_context(tc.tile_pool(name="pst", bufs=8))     # per-tile stats
        psq = ctx.enter_context(tc.tile_pool(name="psq", bufs=2))     # x^2 scratch
        psc = ctx.enter_context(tc.tile_pool(name="psc", bufs=4))     # z-corr temps
        pgt = ctx.enter_context(tc.tile_pool(name="pgt", bufs=4))     # gT tiles
        pout = ctx.enter_context(tc.tile_pool(name="pout", bufs=2))   # out staging
        zps = ctx.enter_context(tc.tile_pool(name="zps", bufs=2, space="PSUM"))
        ztps = ctx.enter_context(tc.tile_pool(name="ztps", bufs=1, space="PSUM"))
        ups = ctx.enter_context(tc.tile_pool(name="ups", bufs=3, space="PSUM"))

        state = {}

        def phase_cast(q):
            """cast + row-sum (one fused ACT Copy w/ accum_out) per tile."""
            xq = xqs[q]
            sumx = pst.tile([P, TPQ], F32, tag="sumx")
            xbs = []
            for i in range(TPQ):
                xb = pxb.tile([P, D], BF16, tag="xb")
                nc.scalar.activation(
                    xb, xq[:, i, :], AF.Copy, accum_out=sumx[:, i : i + 1]
                )
                xbs.append(xb)
            state[q] = (sumx, xbs)

        def phase_a(q):
            """sum-of-squares + stats + transposes for quarter q."""
            sumx, xbs = state[q]
            sumsq = pst.tile([P, TPQ], F32, tag="sumsq")
            xts = []
            for i in range(TPQ):
                x2 = psq.tile([P, D], BF16, tag="x2")
                nc.vector.scalar_tensor_tensor(
                    out=x2,
                    in0=xbs[i],
                    scalar=1.0,
                    in1=xbs[i],
                    op0=OP.mult,
                    op1=OP.mult,
                    accum_out=sumsq[:, i : i + 1],
                )
                # contiguous 2D transpose output (xbar fast path):
                # xt_i[p, c*128 + t] = xb_i[t, c*128 + p]
                xt_i = pxt.tile([P, D], BF16, tag="xt")
                tp_insts.append(
                    nc.sync.dma_start_transpose(
                        out=xt_i.rearrange("p (c t) -> p c t", t=P), in_=xbs[i]
                    )
                )
                xts.append(xt_i)
            # mu = sumx/D ; var = sumsq/D - mu^2 ; rstd = 1/sqrt(var+eps)
            mu_q = pst.tile([P, TPQ], F32, tag="mu")
            nc.vector.tensor_scalar_mul(mu_q, sumx, 1.0 / D)
            musq = pst.tile([P, TPQ], F32, tag="musq")
            nc.vector.tensor_mul(musq, mu_q, mu_q)
            var_q = pst.tile([P, TPQ], F32, tag="var")
            nc.vector.scalar_tensor_tensor(
                out=var_q, in0=sumsq, scalar=1.0 / D, in1=musq,
                op0=OP.mult, op1=OP.subtract,
            )
            srt = pst.tile([P, TPQ], F32, tag="srt")
            nc.scalar.activation(srt, var_q, AF.Sqrt, bias=eps_b)
            rstd_q = pst.tile([P, TPQ], F32, tag="rstd")
            nc.vector.reciprocal(rstd_q, srt)
            mr_q = pst.tile([P, TPQ], F32, tag="mr")
            nc.vector.tensor_mul(mr_q, mu_q, rstd_q)
            state[q] = (xts, rstd_q, mr_q)

        def phase_b(q):
            """matmuls + gelu + residual + store for quarter q."""
            xq = xqs[q]
            xts, rstd_q, mr_q = state.pop(q)
            ot = pout.tile([P, TPQ, D], F32, tag="ot")
            for i in range(TPQ):
                rstd = rstd_q[:, i : i + 1]
                mr = mr_q[:, i : i + 1]
                z = zps.tile([P, BN], F32, tag="z")
                for c in range(NCH):
                    nc.tensor.matmul(
                        z,
                        xts[i][:, c * P : (c + 1) * P],
                        w_sb[:, c, :],
                        start=(c == 0),
                        stop=(c == NCH - 1),
                    )
                # zh = rstd*z + ((-s)*(mu*rstd) + b')
                t3 = psc.tile([P, BN], F32, tag="t3")
                nc.vector.scalar_tensor_tensor(
                    out=t3, in0=sneg_r, scalar=mr, in1=b_rep, op0=OP.mult, op1=OP.add
                )
                zh = psc.tile([P, BN], F32, tag="zh")
                nc.vector.scalar_tensor_tensor(
                    out=zh, in0=z, scalar=rstd, in1=t3, op0=OP.mult, op1=OP.add
                )
                # transpose zh -> [64, 128], gelu into gT rows 0..63, ones row 64
                zt = ztps.tile([BN, P], F32, tag="zt")
                nc.tensor.transpose(zt, zh, ident)
                gt = pgt.tile([BN + 1, P], BF16, tag="gt")
                nc.scalar.activation(gt[0:BN, :], zt, AF.Gelu)
                nc.vector.memset(gt[BN : BN + 1, :], 1.0)
                # up-projection (+ scaled bias via the ones row), then
                # residual out = 1.0*u + x (fp32), per 512-wide half.
                for h in range(2):
                    u = ups.tile([P, H], F32, tag="u")
                    nc.tensor.matmul(
                        u, gt, wue[:, h * H : (h + 1) * H], start=True, stop=True
                    )
                    nc.vector.scalar_tensor_tensor(
                        out=ot[:, i, h * H : (h + 1) * H],
                        in0=u,
                        scalar=1.0,
                        in1=xq[:, i, h * H : (h + 1) * H],
                        op0=OP.mult,
                        op1=OP.add,
                    )
            hq = TPQ // 2
            store_insts.append(
                nc.gpsimd.dma_start(
                    out=out_r[:, q * TPQ : q * TPQ + hq, :], in_=ot[:, 0:hq, :]
                )
            )
            store_insts.append(
                nc.gpsimd.dma_start(
                    out=out_r[:, q * TPQ + hq : (q + 1) * TPQ, :], in_=ot[:, hq:TPQ, :]
                )
            )

        # All casts first (ACT FIFO unblocked), then all stats+transposes
        # (one contiguous xbar window), then all compute/store B phases.
        tp_insts = []
        store_insts = []
        for q in range(NQ):
            phase_cast(q)
        sneg_r, b_rep = preproc_rows()
        for q in range(NQ):
            phase_a(q)
        for q in range(NQ):
            phase_b(q)

        # Force a single xbar window: every transpose after ALL loads, every
        # store after the LAST transpose. Otherwise the scheduler interleaves
        # copies and transposes and every mode transition serializes the
        # SDMA engines.
        for tp in tp_insts:
            for ld in load_insts:
                add_dep_helper(tp.ins, ld.ins, reason="xbar window: after loads")
        for st in store_insts:
            add_dep_helper(st.ins, tp_insts[-1].ins, reason="xbar window: stores after")


_NC = None


def _get_nc():
    global _NC
    if _NC is None:
        _NC = _build_kernel()
    return _NC


def _make_in_maps(inputs):
    x = np.ascontiguousarray(np.asarray(inputs["x"], dtype=np.float32)).reshape(
        TOK_TOTAL, D
    )
    shared = {
        "norm_w": np.ascontiguousarray(np.asarray(inputs["norm_w"], np.float32)),
        "norm_b": np.ascontiguousarray(np.asarray(inputs["norm_b"], np.float32)),
        "w_down": np.ascontiguousarray(np.asarray(inputs["w_down"], np.float32)),
        "b_down": np.ascontiguousarray(np.asarray(inputs["b_down"], np.float32)),
        "w_up": np.ascontiguousarray(np.asarray(inputs["w_up"], np.float32)),
        "b_up": np.ascontiguousarray(np.asarray(inputs["b_up"], np.float32)),
        "scale": np.asarray(inputs["scale"], np.float32).reshape(1, 1),
    }
    in_maps = []
    for c in range(N_CORES):
        m = dict(shared)
        m["x"] = np.ascontiguousarray(x[c * TOK : (c + 1) * TOK])
        in_maps.append(m)
    return in_maps


def run(inputs, trace=False, **kwargs):
    nc = _get_nc()
    in_maps = _make_in_maps(inputs)
    res = bass_utils.run_bass_kernel_spmd(
        nc, in_maps, core_ids=list(range(N_CORES)), trace=trace, **kwargs
    )
    shards = [res.results[c]["out"] for c in range(N_CORES)]
    full = np.concatenate(shards, axis=0).reshape(B, N, D).astype(np.float32)
    return full, res


def kernel(**inputs):
    full, _ = run(inputs, trace=False)
    return full
